# revision 1
# baseline (speedup 1.0000x reference)
"""Trainium2 Bass kernel for nn_BGraphConvolution (BGCN message passing).

v2 design (from baseline trace analysis):
- support-major passes [1,3,2,4,0,5,6]; per pass, dest tiles processed in
  blocks of <=7 tiles; per block, 4 region gathers (sources binned by the
  owner-slice AllGather regions) merged across the block's tiles.
- AllGathers sliced 4x (p during dense phase, d1 during pass3, d2 during
  pass4) so collectives overlap compute.
- edges dest-sorted within (tile, region) bins and packed into 128-slot
  chunks of whole dest-rows; per-chunk one-hot scatter matrices built only
  over the narrow dest window [dlo, dlo+w).
- transposed matmul orientation: lhsT = gathered chunk (128 slots x 128
  feats), rhs = windowed one-hot [128 slots, w] -> psum[feat, dest-window];
  chunk 0 of each tile uses a full-width window with start=True (zero-init).
"""
import os
import numpy as np
import ml_dtypes

N = 100000
D_IN, D_OUT = 256, 128
NCORE = 8
NSH = N // NCORE          # 12500 rows per core
P = 128
NT = (NSH + P - 1) // P   # 98 dest tiles (last has 84 rows)
LAST_ROWS = NSH - (NT - 1) * P
NQ = 4

ROWS_R = [3200, 3200, 3200, 2900]   # owner-local rows per AG slice/region
OFF_R = [0, 3200, 6400, 9600]
GOFF_R = [0, 25600, 51200, 76800]   # region block offsets in p_full
SLICE_T = [(0, 25), (25, 50), (50, 75), (75, 98)]  # dest-tile slices
BMAX = int(os.environ.get("BGCN_BMAX", "1"))
BLOCKS = []          # (slice_idx, t0, t1)
for _si, (_a, _b) in enumerate(SLICE_T):
    _t = _a
    while _t < _b:
        _e = min(_t + BMAX, _b)
        BLOCKS.append((_si, _t, _e))
        _t = _e
PASS_ORDER = [1, 3, 2, 4, 0, 5, 6]
NEED_Q = {1: True, 3: True, 2: True, 4: True, 0: False, 5: False, 6: False}

bf16 = ml_dtypes.bfloat16


def _pack_edges(rl_loc, cols, vals):
    """Per (tile, region): dest-sorted whole-row chunk packing.

    Returns dict[(t, r)] -> (chunks, windows) where chunks is a list of
    (dest_rows_f32[128], idx_i16[128], val_f32[128]) padded slot arrays and
    windows the per-chunk (dlo, dhi_exclusive)."""
    t_arr = rl_loc // P
    d_arr = rl_loc % P
    own = cols // NSH
    loc = cols % NSH
    r_arr = np.minimum(loc // 3200, 3)
    rows_r = np.array(ROWS_R, np.int64)
    off_r = np.array(OFF_R, np.int64)
    idx_arr = own * rows_r[r_arr] + (loc - off_r[r_arr])
    order = np.lexsort((idx_arr, d_arr, r_arr, t_arr))
    t_s, d_s, r_s = t_arr[order], d_arr[order], r_arr[order]
    i_s, v_s = idx_arr[order], vals[order]
    key = (t_s.astype(np.int64) * 4 + r_s)
    out = {}
    bounds = np.searchsorted(key, np.arange(NT * 4 + 1))
    for b in range(NT * 4):
        s, e = bounds[b], bounds[b + 1]
        if s == e:
            continue
        t, r = b // 4, b % 4
        dd, ii, vv = d_s[s:e], i_s[s:e], v_s[s:e]
        rows, counts = np.unique(dd, return_counts=True)
        chunks, windows = [], []
        cur = 0
        cs_d, cs_i, cs_v = [], [], []
        wlo = None
        for row, cnt_ in zip(rows, counts):
            if cur + cnt_ > P and cur > 0:
                chunks.append((cs_d, cs_i, cs_v))
                windows.append((wlo, whi))
                cur = 0
                cs_d, cs_i, cs_v = [], [], []
                wlo = None
            pos = np.searchsorted(dd, row)
            cs_d.append(np.full(cnt_, row, np.float32))
            cs_i.append(ii[pos:pos + cnt_])
            cs_v.append(vv[pos:pos + cnt_])
            if wlo is None:
                wlo = int(row)
            whi = int(row) + 1
            cur += cnt_
        if cur > 0:
            chunks.append((cs_d, cs_i, cs_v))
            windows.append((wlo, whi))
        packed = []
        for cs in chunks:
            dpad = np.zeros(P, np.float32)
            ipad = np.zeros(P, np.int64)
            vpad = np.zeros(P, np.float32)
            dc = np.concatenate(cs[0]); ic = np.concatenate(cs[1]); vc = np.concatenate(cs[2])
            n_ = len(dc)
            dpad[:n_] = dc; ipad[:n_] = ic; vpad[:n_] = vc
            packed.append((dpad, ipad, vpad))
        out[(t, r)] = (packed, windows)
    return out


def _build_meta(rows, cols, vals):
    """Host preprocessing -> shared program meta + per-core packed arrays."""
    vscale = [0.5, 1.0, 1.0, 1.0, 1.0, 0.125, 0.125]
    per = {}
    for m in range(NCORE):
        lo, hi = m * NSH, (m + 1) * NSH
        for s in range(7):
            mask = (rows[s] >= lo) & (rows[s] < hi)
            rl = (rows[s][mask] - lo).astype(np.int64)
            cl = cols[s][mask].astype(np.int64)
            vl = (vals[s][mask] * vscale[s]).astype(np.float32)
            per[(m, s)] = _pack_edges(rl, cl, vl)

    # shared chunk counts + union windows
    cnts = np.zeros((7, NT, 4), np.int32)
    for (m, s), d in per.items():
        for (t, r), (packed, _) in d.items():
            cnts[s, t, r] = max(cnts[s, t, r], len(packed))
    cnts[:, :, 0] = np.maximum(cnts[:, :, 0], 1)  # zero-init chunk must exist
    windows = {}
    for s in range(7):
        for t in range(NT):
            for r in range(4):
                nc_ = cnts[s, t, r]
                lo_ = [128] * nc_
                hi_ = [0] * nc_
                for m in range(NCORE):
                    d = per[(m, s)].get((t, r))
                    if d is None:
                        continue
                    for c, (wl, wh) in enumerate(d[1]):
                        lo_[c] = min(lo_[c], wl)
                        hi_[c] = max(hi_[c], wh)
                ws = []
                for c in range(nc_):
                    if hi_[c] <= lo_[c]:
                        ws.append((0, 4))
                    else:
                        dlo = lo_[c] & ~3
                        w = -(-(hi_[c] - dlo) // 4) * 4
                        ws.append((dlo, min(w, P - dlo)))
                windows[(s, t, r)] = ws

    # segment offsets (shared): order = PASS_ORDER x BLOCKS x regions
    off_idx, off_ch = {}, {}
    io = co = 0
    for s in PASS_ORDER:
        for bi, (_si, t0, t1) in enumerate(BLOCKS):
            for r in range(4):
                off_idx[(s, bi, r)] = io
                off_ch[(s, bi, r)] = co
                w = int(cnts[s, t0:t1, r].sum())
                io += w * 8
                co += w
    idx_w, ch_w = io, co

    # per-core packed arrays
    in_meta = []
    for m in range(NCORE):
        idx_all = np.zeros((P, idx_w), np.int16)
        rlv_all = np.zeros((P, 2 * ch_w), np.float32)
        for s in PASS_ORDER:
            for bi, (_si, t0, t1) in enumerate(BLOCKS):
                for r in range(4):
                    CNT = int(cnts[s, t0:t1, r].sum())
                    if CNT == 0:
                        continue
                    iseg = np.zeros(CNT * P, np.int16)
                    rseg = np.zeros(CNT * P, np.float32)
                    vseg = np.zeros(CNT * P, np.float32)
                    coff = 0
                    for t in range(t0, t1):
                        d = per[(m, s)].get((t, r))
                        packed = d[0] if d else []
                        for c in range(cnts[s, t, r]):
                            if c < len(packed):
                                dpad, ipad, vpad = packed[c]
                                fl = (coff + c) * P
                                iseg[fl:fl + P] = ipad.astype(np.int16)
                                rseg[fl:fl + P] = dpad
                                vseg[fl:fl + P] = vpad
                        coff += cnts[s, t, r]
                    io0 = off_idx[(s, bi, r)]
                    co0 = off_ch[(s, bi, r)]
                    idx_all[:, io0:io0 + CNT * 8] = np.tile(
                        iseg.reshape(CNT * 8, 16).T, (8, 1))
                    rlv_all[:, co0:co0 + CNT] = rseg.reshape(CNT, P).T
                    rlv_all[:, ch_w + co0:ch_w + co0 + CNT] = vseg.reshape(CNT, P).T
        in_meta.append((idx_all, rlv_all))
    return cnts, windows, off_idx, off_ch, idx_w, ch_w, in_meta


def _build_program(cnts, windows, off_idx, off_ch, idx_w, ch_w):
    import concourse.bass as bass
    import concourse.tile as tile
    from concourse import bacc, mybir, library_config
    from concourse.masks import make_identity
    from contextlib import ExitStack

    fp32 = mybir.dt.float32
    bft = mybir.dt.bfloat16
    KCH = D_IN // P  # 2
    AluOp = mybir.AluOpType
    ActFn = mybir.ActivationFunctionType

    # max chunk count per (block, region) gather -> G/Q buffer geometry
    cnt_br = {}
    for s in range(7):
        for bi, (_si, t0, t1) in enumerate(BLOCKS):
            for r in range(4):
                cnt_br[(s, bi, r)] = int(cnts[s, t0:t1, r].sum())
    MAXC = max(cnt_br.values())

    nc = bacc.Bacc("TRN2", target_bir_lowering=False, debug=False,
                   num_devices=NCORE, num_swdge_queues=NQ,
                   dynamic_dma_scratch_size=49152)
    xt_d = nc.dram_tensor("xt", [D_IN, NSH], fp32, kind="ExternalInput").ap()
    wa_d = nc.dram_tensor("wa", [D_IN, D_OUT], fp32, kind="ExternalInput").ap()
    wb_d = nc.dram_tensor("wb", [D_IN, D_OUT], fp32, kind="ExternalInput").ap()
    w1_d = nc.dram_tensor("w1", [D_OUT, 32], fp32, kind="ExternalInput").ap()
    b1_d = nc.dram_tensor("b1", [1, 32], fp32, kind="ExternalInput").ap()
    w2_d = nc.dram_tensor("w2", [32, 1], fp32, kind="ExternalInput").ap()
    iota_d = nc.dram_tensor("iota", [P, P], fp32, kind="ExternalInput").ap()
    idx_d = nc.dram_tensor("idxm", [P, idx_w], mybir.dt.int16,
                           kind="ExternalInput").ap()
    rlv_d = nc.dram_tensor("rlvm", [P, 2 * ch_w], fp32, kind="ExternalInput").ap()
    out_d = nc.dram_tensor("out", [NSH, D_OUT], fp32, kind="ExternalOutput").ap()

    qctr = [0]

    def next_q():
        q = qctr[0] % NQ
        qctr[0] += 1
        return q

    rg = [list(range(NCORE))]

    with tile.TileContext(nc) as tc, ExitStack() as ctx:
        const_pool = ctx.enter_context(tc.tile_pool(name="const", bufs=1))
        persist = ctx.enter_context(tc.tile_pool(name="persist", bufs=1))
        meta_pool = ctx.enter_context(tc.tile_pool(name="meta", bufs=3))
        g_pool = ctx.enter_context(tc.tile_pool(name="g", bufs=2))
        q_pool = ctx.enter_context(tc.tile_pool(name="q", bufs=1))
        s_pool = ctx.enter_context(tc.tile_pool(name="s", bufs=8))
        o_pool = ctx.enter_context(tc.tile_pool(name="o", bufs=4))
        dram = ctx.enter_context(tc.tile_pool(name="dram", bufs=1, space="DRAM"))

        nc.gpsimd.load_library(library_config.mlp)

        iota_t = const_pool.tile([P, P], fp32)
        nc.sync.dma_start(iota_t[:], iota_d[:])
        ident = const_pool.tile([P, P], fp32)
        make_identity(nc, ident[:])
        wa_t = const_pool.tile([P, KCH * D_OUT], fp32, tag="wa")
        wb_t = const_pool.tile([P, KCH * D_OUT], fp32, tag="wb")
        for k in range(KCH):
            nc.sync.dma_start(wa_t[:, k * D_OUT:(k + 1) * D_OUT],
                              wa_d[k * P:(k + 1) * P, :])
            nc.sync.dma_start(wb_t[:, k * D_OUT:(k + 1) * D_OUT],
                              wb_d[k * P:(k + 1) * P, :])
        w1_t = const_pool.tile([P, 32], fp32)
        nc.sync.dma_start(w1_t[:], w1_d[:])
        b1_t = const_pool.tile([1, 32], fp32)
        nc.sync.dma_start(b1_t[:], b1_d[:])
        w2_t = const_pool.tile([32, 1], fp32)
        nc.sync.dma_start(w2_t[:], w2_d[:])
        ones_t = const_pool.tile([1, P], fp32)
        nc.vector.memset(ones_t[:], 1.0)
        zlhs_t = const_pool.tile([P, P], bft, tag="zlhs")
        nc.vector.memset(zlhs_t[:], 0.0)
        zrhs_t = const_pool.tile([P, 4 * P], bft, tag="zrhs")
        nc.vector.memset(zrhs_t[:], 0.0)

        b_store = persist.tile([P, NT * P], bft, tag="bstore")
        acc_store = persist.tile([P, NT * P], bft, tag="accstore")

        p_local = dram.tile([NSH, D_OUT], bft, tag="p_local")
        d1_local = dram.tile([NSH, D_OUT], bft, tag="d1_local")
        d2_local = dram.tile([NSH, D_OUT], bft, tag="d2_local")
        p_full = [dram.tile([8 * ROWS_R[r], D_OUT], bft, tag=f"p_full{r}",
                            name=f"p_full{r}", addr_space="Shared")
                  for r in range(4)]
        d1_full = [dram.tile([8 * ROWS_R[r], D_OUT], bft, tag=f"d1_full{r}",
                             name=f"d1_full{r}", addr_space="Shared")
                   for r in range(4)]
        d2_full = [dram.tile([8 * ROWS_R[r], D_OUT], bft, tag=f"d2_full{r}",
                             name=f"d2_full{r}", addr_space="Shared")
                   for r in range(4)]

        def ag_slice(local, full, r):
            nc.gpsimd.collective_compute(
                "AllGather", mybir.AluOpType.bypass, replica_groups=rg,
                ins=[local[OFF_R[r]:OFF_R[r] + ROWS_R[r], :]],
                outs=[full[r][:, :]])

        # ---------- dense phase: pre_sup (feature-major) ----------
        with tc.tile_pool(name="dense", bufs=2) as dense_pool, \
             tc.tile_pool(name="dpsum", bufs=1, space="PSUM") as dpsum_pool:
            for t in range(NT):
                rows = P if t < NT - 1 else LAST_ROWS
                sl = slice(t * P, t * P + rows)
                xt_t = dense_pool.tile([P, KCH * P], fp32, tag="xt")
                for k in range(KCH):
                    nc.sync.dma_start(xt_t[:, k * P:k * P + rows],
                                      xt_d[k * P:(k + 1) * P, sl])
                psa = dpsum_pool.tile([P, P], fp32, tag="pa")
                psb = dpsum_pool.tile([P, P], fp32, tag="pb")
                for k in range(KCH):
                    nc.tensor.matmul(psa[:, :rows],
                                     lhsT=wa_t[:, k * D_OUT:(k + 1) * D_OUT],
                                     rhs=xt_t[:, k * P:k * P + rows],
                                     start=(k == 0), stop=(k == KCH - 1))
                    nc.tensor.matmul(psb[:, :rows],
                                     lhsT=wb_t[:, k * D_OUT:(k + 1) * D_OUT],
                                     rhs=xt_t[:, k * P:k * P + rows],
                                     start=(k == 0), stop=(k == KCH - 1))
                a_sb = dense_pool.tile([P, P], fp32, tag="a_sb")
                nc.vector.tensor_copy(a_sb[:, :rows], psa[:, :rows])
                tmp = dense_pool.tile([P, P], fp32, tag="tmp")
                nc.vector.tensor_tensor(out=tmp[:, :rows], in0=a_sb[:, :rows],
                                        in1=psb[:, :rows], op=AluOp.subtract)
                nc.vector.tensor_tensor(out=tmp[:, :rows], in0=tmp[:, :rows],
                                        in1=a_sb[:, :rows], op=AluOp.mult)
                al_sb = dense_pool.tile([P, P], fp32, tag="al_sb")
                nc.vector.tensor_scalar(out=al_sb[:, :rows], in0=tmp[:, :rows],
                                        scalar1=0.5, scalar2=None,
                                        op0=AluOp.mult)
                nc.vector.tensor_tensor(out=al_sb[:, :rows],
                                        in0=al_sb[:, :rows],
                                        in1=a_sb[:, :rows], op=AluOp.add)
                z = []
                for zi, comp_sb in enumerate((a_sb, al_sb)):
                    psh = dpsum_pool.tile([32, P], fp32, tag="ph")
                    nc.tensor.matmul(psh[:, :rows], lhsT=w1_t[:],
                                     rhs=comp_sb[:, :rows],
                                     start=True, stop=False)
                    nc.tensor.matmul(psh[:, :rows], lhsT=b1_t[:],
                                     rhs=ones_t[:, :rows],
                                     start=False, stop=True)
                    h_sb = dense_pool.tile([32, P], fp32, tag="h_sb")
                    nc.scalar.activation(h_sb[:, :rows], psh[:, :rows],
                                         ActFn.Tanh)
                    psz = dpsum_pool.tile([1, P], fp32, tag="pz")
                    nc.tensor.matmul(psz[:, :rows], lhsT=w2_t[:],
                                     rhs=h_sb[:, :rows], start=True, stop=True)
                    z_sb = dense_pool.tile([1, P], fp32, tag=f"z{zi}")
                    nc.vector.tensor_copy(z_sb[:, :rows], psz[:, :rows])
                    z.append(z_sb)
                dz = dense_pool.tile([1, P], fp32, tag="dz")
                nc.vector.tensor_tensor(out=dz[:, :rows], in0=z[1][:, :rows],
                                        in1=z[0][:, :rows], op=AluOp.subtract)
                ez = dense_pool.tile([1, P], fp32, tag="ez")
                nc.scalar.activation(ez[:, :rows], dz[:, :rows], ActFn.Exp)
                nc.vector.tensor_scalar(out=ez[:, :rows], in0=ez[:, :rows],
                                        scalar1=1.0, scalar2=None,
                                        op0=AluOp.add)
                atta = dense_pool.tile([1, P], fp32, tag="atta")
                nc.vector.reciprocal(atta[:, :rows], ez[:, :rows])
                # broadcast att along partitions via K=1 matmul (PE, not gpsimd)
                attps = dpsum_pool.tile([P, P], fp32, tag="attps")
                nc.tensor.matmul(attps[:, :rows], lhsT=ones_t[:],
                                 rhs=atta[:, :rows], start=True, stop=True)
                t1 = dense_pool.tile([P, P], fp32, tag="t1")
                nc.vector.tensor_tensor(out=t1[:, :rows], in0=a_sb[:, :rows],
                                        in1=attps[:, :rows], op=AluOp.mult)
                # pst = t1 + al - al*att = att*a + (1-att)*al
                t2 = dense_pool.tile([P, P], fp32, tag="t2")
                nc.vector.tensor_tensor(out=t2[:, :rows], in0=al_sb[:, :rows],
                                        in1=attps[:, :rows], op=AluOp.mult)
                pst = dense_pool.tile([P, P], fp32, tag="pst")
                nc.vector.tensor_tensor(out=pst[:, :rows], in0=al_sb[:, :rows],
                                        in1=t2[:, :rows], op=AluOp.subtract)
                nc.vector.tensor_tensor(out=pst[:, :rows], in0=pst[:, :rows],
                                        in1=t1[:, :rows], op=AluOp.add)
                ptp = dpsum_pool.tile([P, P], fp32, tag="ptp")
                nc.tensor.transpose(out=ptp[:rows, :], in_=pst[:, :rows],
                                    identity=ident[:])
                prow = dense_pool.tile([P, P], bft, tag="prow")
                nc.vector.tensor_copy(prow[:rows, :], ptp[:rows, :])
                nc.sync.dma_start(p_local[sl, :], prow[:rows, :])
                for r in range(4):
                    if t == SLICE_T[r][1] - 1:
                        ag_slice(p_local, p_full, r)

        # ---------- SpMM passes ----------
        def run_pass(s, src_full, sink):
            need_q = NEED_Q[s]
            with tc.tile_pool(name=f"ps{s}", bufs=2, space="PSUM") as psp, \
                 tc.tile_pool(name=f"tp{s}", bufs=2, space="PSUM") as tpp:
                for bi, (si_, t0, t1) in enumerate(BLOCKS):
                    nt_b = t1 - t0
                    psA = psp.tile([P, 4 * P], fp32, tag="sacc",
                                   name="psA")
                    pss = [psA[:, (t - t0) * P:(t - t0 + 1) * P]
                           for t in range(t0, t1)]
                    psq = None
                    nc.tensor.matmul(psA[:, :], lhsT=zlhs_t[:], rhs=zrhs_t[:],
                                     start=True, stop=False,
                                     skip_group_check=True)
                    if need_q:
                        psB_ = psp.tile([P, 4 * P], fp32, tag="qacc",
                                        name="psB_")
                        psq = [psB_[:, (t - t0) * P:(t - t0 + 1) * P]
                               for t in range(t0, t1)]
                        nc.tensor.matmul(psB_[:, :], lhsT=zlhs_t[:],
                                         rhs=zrhs_t[:], start=True, stop=False,
                                         skip_group_check=True)
                    # last (region, chunk) per tile for stop flag
                    lastrc = {}
                    for t in range(t0, t1):
                        for r in range(4):
                            if cnts[s, t, r] > 0:
                                lastrc[t] = (r, cnts[s, t, r] - 1)
                    for r in range(4):
                        CNT = cnt_br[(s, bi, r)]
                        if CNT == 0:
                            continue
                        io0 = off_idx[(s, bi, r)]
                        co0 = off_ch[(s, bi, r)]
                        idxt = meta_pool.tile([P, MAXC * 8], mybir.dt.int16,
                                              tag="idxt")
                        rlvt = meta_pool.tile([P, 2 * MAXC], fp32, tag="rlvt")
                        nc.sync.dma_start(idxt[:, :CNT * 8],
                                          idx_d[:, io0:io0 + CNT * 8])
                        nc.sync.dma_start(rlvt[:, :CNT],
                                          rlv_d[:, co0:co0 + CNT])
                        nc.sync.dma_start(rlvt[:, MAXC:MAXC + CNT],
                                          rlv_d[:, ch_w + co0:ch_w + co0 + CNT])
                        gt = g_pool.tile([P, MAXC * P], bft, tag="G")
                        g3 = gt[:].rearrange("p (c d) -> p c d", d=P)
                        nc.gpsimd.dma_gather(
                            out_ap=g3[:, 0:CNT, :],
                            in_ap=src_full[r][:, :],
                            idxs_ap=idxt[:, 0:CNT * 8],
                            num_idxs=CNT * P, num_idxs_reg=CNT * P,
                            elem_size=D_OUT, single_packet=False,
                            queue_num=next_q(),
                        )
                        if need_q:
                            qt = q_pool.tile([P, MAXC * P], bft, tag="Q")
                            q3 = qt[:].rearrange("p (c d) -> p c d", d=P)
                        coff = 0
                        for t in range(t0, t1):
                            nch = int(cnts[s, t, r])
                            if nch == 0:
                                continue
                            if need_q:
                                nc.scalar.square(
                                    qt[:, coff * P:(coff + nch) * P],
                                    gt[:, coff * P:(coff + nch) * P])
                            ws = windows[(s, t, r)]
                            for c in range(nch):
                                dlo, w = ws[c]
                                st_t = s_pool.tile([P, P], bft)
                                nc.vector.tensor_scalar(
                                    out=st_t[:, :w], in0=iota_t[:, dlo:dlo + w],
                                    scalar1=rlvt[:, coff + c:coff + c + 1],
                                    scalar2=rlvt[:, MAXC + coff + c:MAXC + coff + c + 1],
                                    op0=AluOp.is_equal, op1=AluOp.mult)
                                last = lastrc.get(t) == (r, c)
                                nc.tensor.matmul(
                                    pss[t - t0][:, dlo:dlo + w],
                                    lhsT=g3[:, coff + c, :], rhs=st_t[:, :w],
                                    start=False, stop=last,
                                    skip_group_check=True)
                                if need_q:
                                    nc.tensor.matmul(
                                        psq[t - t0][:, dlo:dlo + w],
                                        lhsT=q3[:, coff + c, :], rhs=st_t[:, :w],
                                        start=False, stop=last,
                                        skip_group_check=True)
                            coff += nch
                    for t in range(t0, t1):
                        sink(t, pss[t - t0], psq[t - t0] if need_q else None,
                             tpp)

        def sink_b(t, ps, pq, tpp):
            """b_store[:, tile] = s^2 - q (bf16, feature-major)."""
            sq = o_pool.tile([P, P], fp32, tag="sq")
            nc.scalar.activation(sq[:], ps[:], ActFn.Square)
            nc.vector.tensor_tensor(out=b_store[:, t * P:(t + 1) * P],
                                    in0=sq[:], in1=pq[:], op=AluOp.subtract)

        def mk_sink_d(dloc, dfull):
            done = [0]

            def sink_d(t, ps, pq, tpp):
                rows = P if t < NT - 1 else LAST_ROWS
                sq = o_pool.tile([P, P], fp32, tag="sq")
                nc.scalar.activation(sq[:], ps[:], ActFn.Square)
                bb = o_pool.tile([P, P], fp32, tag="bb")
                nc.vector.tensor_tensor(out=bb[:], in0=sq[:], in1=pq[:],
                                        op=AluOp.subtract)
                dT = o_pool.tile([P, P], fp32, tag="dT")
                nc.vector.tensor_tensor(out=dT[:], in0=b_store[:, t * P:(t + 1) * P],
                                        in1=bb[:], op=AluOp.subtract)
                ptp = tpp.tile([P, P], fp32, tag="ptp")
                nc.tensor.transpose(out=ptp[:rows, :], in_=dT[:, :rows],
                                    identity=ident[:])
                drow = o_pool.tile([P, P], bft, tag="drow")
                nc.vector.tensor_copy(drow[:rows, :], ptp[:rows, :])
                nc.sync.dma_start(dloc[t * P:t * P + rows, :],
                                  drow[:rows, :])
                for r in range(4):
                    if t == SLICE_T[r][1] - 1:
                        ag_slice(dloc, dfull, r)
            return sink_d

        def sink_acc0(t, ps, pq, tpp):
            nc.vector.tensor_copy(acc_store[:, t * P:(t + 1) * P], ps[:])

        def sink_acc5(t, ps, pq, tpp):
            nc.vector.tensor_tensor(out=acc_store[:, t * P:(t + 1) * P],
                                    in0=acc_store[:, t * P:(t + 1) * P],
                                    in1=ps[:], op=AluOp.add)

        def sink_out(t, ps, pq, tpp):
            rows = P if t < NT - 1 else LAST_ROWS
            oT = o_pool.tile([P, P], fp32, tag="oT")
            nc.vector.tensor_tensor(out=oT[:], in0=ps[:],
                                    in1=acc_store[:, t * P:(t + 1) * P],
                                    op=AluOp.add)
            oR = o_pool.tile([P, P], fp32, tag="oR")
            nc.scalar.activation(oR[:], oT[:], ActFn.Relu)
            ptp = tpp.tile([P, P], fp32, tag="ptpo")
            nc.tensor.transpose(out=ptp[:rows, :], in_=oR[:, :rows],
                                identity=ident[:])
            orow = o_pool.tile([P, P], fp32, tag="orow")
            nc.vector.tensor_copy(orow[:rows, :], ptp[:rows, :])
            nc.sync.dma_start(out_d[t * P:t * P + rows, :], orow[:rows, :])

        def sink_out0(t, ps, pq, tpp):
            rows = P if t < NT - 1 else LAST_ROWS
            oR = o_pool.tile([P, P], fp32, tag="oR")
            nc.scalar.activation(oR[:], ps[:], ActFn.Relu)
            ptp = tpp.tile([P, P], fp32, tag="ptpo")
            nc.tensor.transpose(out=ptp[:rows, :], in_=oR[:, :rows],
                                identity=ident[:])
            orow = o_pool.tile([P, P], fp32, tag="orow")
            nc.vector.tensor_copy(orow[:rows, :], ptp[:rows, :])
            nc.sync.dma_start(out_d[t * P:t * P + rows, :], orow[:rows, :])

        if os.environ.get("BGCN_P0"):
            run_pass(0, p_full, sink_out0)
        else:
            run_pass(1, p_full, sink_b)
            run_pass(3, p_full, mk_sink_d(d1_local, d1_full))
            run_pass(2, p_full, sink_b)
            run_pass(4, p_full, mk_sink_d(d2_local, d2_full))
            run_pass(0, p_full, sink_acc0)
            run_pass(5, d1_full, sink_acc5)
            run_pass(6, d2_full, sink_out)

    nc.compile()
    return nc


def kernel(x, Wa, Wb, Wc, attn_w1, attn_b1, attn_w2, rows, cols, vals):
    from concourse.bass_utils import run_bass_kernel_spmd

    x = np.asarray(x, np.float32)
    Wa = np.asarray(Wa, np.float32)
    Wb = np.asarray(Wb, np.float32)
    attn_w1 = np.asarray(attn_w1, np.float32)
    attn_b1 = np.asarray(attn_b1, np.float32)
    attn_w2 = np.asarray(attn_w2, np.float32)
    rows = np.asarray(rows)
    cols = np.asarray(cols)
    vals = np.asarray(vals, np.float32)

    cnts, windows, off_idx, off_ch, idx_w, ch_w, in_meta = _build_meta(
        rows, cols, vals)

    iota_np = np.ascontiguousarray(np.tile(np.arange(P, dtype=np.float32), (P, 1)))
    in_maps = []
    for m in range(NCORE):
        idx_all, rlv_all = in_meta[m]
        xt = np.ascontiguousarray(x[m * NSH:(m + 1) * NSH, :].T)
        in_maps.append({
            "xt": xt, "wa": Wa, "wb": Wb, "w1": attn_w1,
            "b1": attn_b1.reshape(1, 32), "w2": attn_w2, "iota": iota_np,
            "idxm": idx_all, "rlvm": rlv_all,
        })

    nc = _build_program(cnts, windows, off_idx, off_ch, idx_w, ch_w)
    res = run_bass_kernel_spmd(nc, in_maps, core_ids=list(range(NCORE)))
    out = np.concatenate([res.results[m]["out"] for m in range(NCORE)], axis=0)
    return np.ascontiguousarray(out.astype(np.float32))



# revision 5
# speedup vs baseline: 1.2175x; 1.2175x over previous
"""Trainium2 Bass kernel for nn_BGraphConvolution (BGCN message passing).

v3 design (from v2 trace analysis: Pool engine was the critical path at 92%
coverage; dma_gather descriptor-gen is ~2.5ns/idx intrinsic and per-index
bound; DVE per-chunk one-hot builds and their port contention with GpSimd
made everything 3x worse):
- scatter matrices (val-scaled one-hots) precomputed on HOST and DMA'd in;
  no DVE work per chunk.
- flipped matmul orientation: lhsT = scatter chunk [128 slots, w dests]
  (stationary, cheap LDWEIGHTS), rhs = gathered chunk (streaming). Output
  psum is dest-row-major -> no output transposes.
- dense phase computes pq = [p | p^2] concatenated [N, 256]; passes 0-4
  gather 512B elements (free: gather cost is per-index), so one N=256
  matmul per chunk produces both the s-SpMM and q-SpMM halves. No ACT
  squares in the SpMM loop.
- dense chunk packing (chunks of 128 edge slots, dest-sorted, rows may
  split across chunks), PSUM windows aligned per PE tile_position rules.
- AllGathers sliced 4x, overlapped with the dense phase / d-pass sinks.
"""
import os
import numpy as np
import ml_dtypes

N = 100000
D_IN, D_OUT = 256, 128
NCORE = 8
NSH = N // NCORE          # 12500 rows per core
P = 128
NT = (NSH + P - 1) // P   # 98 dest tiles (last has 84 rows)
LAST_ROWS = NSH - (NT - 1) * P
NQ = 4

ROWS_R = [3200, 3200, 3200, 2900]   # owner-local rows per AG slice/region
OFF_R = [0, 3200, 6400, 9600]
SLICE_T = [(0, 25), (25, 50), (50, 75), (75, 98)]  # dest-tile slices
BMAX = int(os.environ.get("BGCN_BMAX", "4"))
BLOCKS = []          # (slice_idx, t0, t1)
for _si, (_a, _b) in enumerate(SLICE_T):
    _t = _a
    while _t < _b:
        _e = min(_t + BMAX, _b)
        BLOCKS.append((_si, _t, _e))
        _t = _e
PASS_ORDER = [1, 3, 2, 4, 0, 5, 6]
NEED_Q = {1: True, 3: True, 2: True, 4: True, 0: False, 5: False, 6: False}
VSCALE = [0.5, 1.0, 1.0, 1.0, 1.0, 0.125, 0.125]

bf16 = ml_dtypes.bfloat16


def _aligned_window(dmin, dmax):
    """Smallest PE-legal psum window [dlo, dlo+w) covering [dmin, dmax).

    Legal: w<=32 at dlo in {0,32,64,96}; w<=64 at dlo in {0,64}; else dlo=0.
    """
    b32 = dmin // 32
    if dmax <= (b32 + 1) * 32:
        return b32 * 32, dmax - b32 * 32
    b64 = dmin // 64
    if dmax <= (b64 + 1) * 64:
        return b64 * 64, dmax - b64 * 64
    return 0, dmax


def _build_meta(rows, cols, vals):
    """Host preprocessing.

    Returns (cnts, windows, off_idx, off_st, idx_w, st_w, in_meta) where
    cnts[s,t,r] = shared chunk count, windows[(s,t,r)] = [(dlo,w), ...],
    and in_meta[m] = (idx_all [128, idx_w] int16, st_all [128, st_w] bf16).
    """
    rows_r = np.array(ROWS_R, np.int64)
    off_r = np.array(OFF_R, np.int64)

    # per (m, s): sorted edge arrays + per-(t,r) boundaries
    per = {}
    for m in range(NCORE):
        lo, hi = m * NSH, (m + 1) * NSH
        for s in range(7):
            mask = (rows[s] >= lo) & (rows[s] < hi)
            rl = (rows[s][mask] - lo).astype(np.int64)
            cl = cols[s][mask].astype(np.int64)
            vl = (vals[s][mask] * VSCALE[s]).astype(np.float32)
            t_arr = rl // P
            d_arr = rl % P
            own = cl // NSH
            loc = cl % NSH
            r_arr = np.minimum(loc // 3200, 3)
            idx_arr = own * rows_r[r_arr] + (loc - off_r[r_arr])
            order = np.lexsort((idx_arr, d_arr, r_arr, t_arr))
            key = t_arr[order] * 4 + r_arr[order]
            bounds = np.searchsorted(key, np.arange(NT * 4 + 1))
            per[(m, s)] = (d_arr[order], idx_arr[order], vl[order], bounds)

    # shared chunk counts
    cnts = np.zeros((7, NT, 4), np.int32)
    for (m, s), (_, _, _, bounds) in per.items():
        ne = bounds[1:] - bounds[:-1]
        nch = -(-ne // P)
        cnts[s] = np.maximum(cnts[s], nch.reshape(NT, 4))
    cnts = np.maximum(cnts, 1)  # keep zero-init/window structure simple

    # shared windows: union over cores of each chunk's dest span
    windows = {}
    for s in range(7):
        for t in range(NT):
            for r in range(4):
                nc_ = int(cnts[s, t, r])
                lo_ = np.full(nc_, P, np.int64)
                hi_ = np.zeros(nc_, np.int64)
                for m in range(NCORE):
                    d_s, _, _, bounds = per[(m, s)]
                    b0, b1 = bounds[t * 4 + r], bounds[t * 4 + r + 1]
                    if b0 == b1:
                        continue
                    dd = d_s[b0:b1]
                    for c in range(-(-(b1 - b0) // P)):
                        seg = dd[c * P:(c + 1) * P]
                        lo_[c] = min(lo_[c], int(seg[0]))
                        hi_[c] = max(hi_[c], int(seg[-1]) + 1)
                ws = []
                for c in range(nc_):
                    if hi_[c] <= lo_[c]:
                        ws.append((0, 4))
                    else:
                        ws.append(_aligned_window(int(lo_[c]), int(hi_[c])))
                windows[(s, t, r)] = ws

    # shared segment offsets: order = PASS_ORDER x BLOCKS x regions
    off_idx, off_st = {}, {}
    io = so = 0
    for s in PASS_ORDER:
        for bi, (_si, t0, t1) in enumerate(BLOCKS):
            for r in range(4):
                off_idx[(s, bi, r)] = io
                off_st[(s, bi, r)] = so
                io += int(cnts[s, t0:t1, r].sum()) * 8
                for t in range(t0, t1):
                    for (_dlo, w) in windows[(s, t, r)]:
                        so += w
    idx_w, st_w = io, so

    # per-core packed arrays
    in_meta = []
    for m in range(NCORE):
        idx_all = np.zeros((P, idx_w), np.int16)
        st_all = np.zeros((P, st_w), bf16)
        st_f32 = np.zeros((P, st_w), np.float32)
        for s in PASS_ORDER:
            d_s, i_s, v_s, bounds = per[(m, s)]
            for bi, (_si, t0, t1) in enumerate(BLOCKS):
                for r in range(4):
                    CNT = int(cnts[s, t0:t1, r].sum())
                    iseg = np.zeros(CNT * P, np.int16)
                    coff = 0
                    soff = off_st[(s, bi, r)]
                    for t in range(t0, t1):
                        b0, b1 = bounds[t * 4 + r], bounds[t * 4 + r + 1]
                        ne = b1 - b0
                        nch = int(cnts[s, t, r])
                        if ne > 0:
                            iseg[coff * P:coff * P + ne] = i_s[b0:b1]
                            ws = windows[(s, t, r)]
                            # scatter vals into st columns
                            e = np.arange(ne)
                            ch = e // P
                            slot = e % P
                            dlo_arr = np.array([ws[c][0] for c in ch])
                            w_cum = np.concatenate(
                                ([0], np.cumsum([w for (_dl, w) in ws])))
                            col = soff + w_cum[ch] + (d_s[b0:b1] - dlo_arr)
                            st_f32[slot, col] = v_s[b0:b1]
                        coff += nch
                        soff += sum(w for (_dl, w) in windows[(s, t, r)])
                    io0 = off_idx[(s, bi, r)]
                    idx_all[:, io0:io0 + CNT * 8] = np.tile(
                        iseg.reshape(CNT * 8, 16).T, (8, 1))
        st_all[:] = st_f32.astype(bf16)
        in_meta.append((idx_all, st_all))
    return cnts, windows, off_idx, off_st, idx_w, st_w, in_meta


def _build_program(cnts, windows, off_idx, off_st, idx_w, st_w):
    import concourse.bass as bass
    import concourse.tile as tile
    from concourse import bacc, mybir, library_config
    from concourse.masks import make_identity
    from contextlib import ExitStack

    fp32 = mybir.dt.float32
    bft = mybir.dt.bfloat16
    KCH = D_IN // P  # 2
    AluOp = mybir.AluOpType
    ActFn = mybir.ActivationFunctionType

    cnt_br = {}
    for s in range(7):
        for bi, (_si, t0, t1) in enumerate(BLOCKS):
            for r in range(4):
                cnt_br[(s, bi, r)] = int(cnts[s, t0:t1, r].sum())
    MAXC = max(cnt_br.values())
    stw_br = {}
    for s in range(7):
        for bi, (_si, t0, t1) in enumerate(BLOCKS):
            for r in range(4):
                stw_br[(s, bi, r)] = sum(
                    w for t in range(t0, t1) for (_dl, w) in windows[(s, t, r)])
    MAXW = max(stw_br.values())

    nc = bacc.Bacc("TRN2", target_bir_lowering=False, debug=False,
                   num_devices=NCORE, num_swdge_queues=NQ,
                   dynamic_dma_scratch_size=49152)
    xt_d = nc.dram_tensor("xt", [D_IN, NSH], fp32, kind="ExternalInput").ap()
    wa_d = nc.dram_tensor("wa", [D_IN, D_OUT], fp32, kind="ExternalInput").ap()
    wb_d = nc.dram_tensor("wb", [D_IN, D_OUT], fp32, kind="ExternalInput").ap()
    w1_d = nc.dram_tensor("w1", [D_OUT, 32], fp32, kind="ExternalInput").ap()
    b1_d = nc.dram_tensor("b1", [1, 32], fp32, kind="ExternalInput").ap()
    w2_d = nc.dram_tensor("w2", [32, 1], fp32, kind="ExternalInput").ap()
    idx_d = nc.dram_tensor("idxm", [P, idx_w], mybir.dt.int16,
                           kind="ExternalInput").ap()
    st_d = nc.dram_tensor("stm", [P, st_w], bft, kind="ExternalInput").ap()
    out_d = nc.dram_tensor("out", [NSH, D_OUT], fp32, kind="ExternalOutput").ap()

    qctr = [0]

    def next_q():
        q = qctr[0] % NQ
        qctr[0] += 1
        return q

    rg = [list(range(NCORE))]

    with tile.TileContext(nc) as tc, ExitStack() as ctx:
        const_pool = ctx.enter_context(tc.tile_pool(name="const", bufs=1))
        persist = ctx.enter_context(tc.tile_pool(name="persist", bufs=1))
        meta_pool = ctx.enter_context(tc.tile_pool(name="meta", bufs=4))
        g_pool = ctx.enter_context(tc.tile_pool(name="g", bufs=2))
        o_pool = ctx.enter_context(tc.tile_pool(name="o", bufs=6))
        dram = ctx.enter_context(tc.tile_pool(name="dram", bufs=1, space="DRAM"))

        nc.gpsimd.load_library(library_config.mlp)

        ident = const_pool.tile([P, P], fp32)
        make_identity(nc, ident[:])
        wa_t = const_pool.tile([P, KCH * D_OUT], fp32, tag="wa")
        wb_t = const_pool.tile([P, KCH * D_OUT], fp32, tag="wb")
        for k in range(KCH):
            nc.sync.dma_start(wa_t[:, k * D_OUT:(k + 1) * D_OUT],
                              wa_d[k * P:(k + 1) * P, :])
            nc.sync.dma_start(wb_t[:, k * D_OUT:(k + 1) * D_OUT],
                              wb_d[k * P:(k + 1) * P, :])
        w1_t = const_pool.tile([P, 32], fp32)
        nc.sync.dma_start(w1_t[:], w1_d[:])
        b1_t = const_pool.tile([1, 32], fp32)
        nc.sync.dma_start(b1_t[:], b1_d[:])
        w2_t = const_pool.tile([32, 1], fp32)
        nc.sync.dma_start(w2_t[:], w2_d[:])
        ones_t = const_pool.tile([1, P], fp32)
        nc.vector.memset(ones_t[:], 1.0)
        zlhs_t = const_pool.tile([P, P], bft, tag="zlhs")
        nc.vector.memset(zlhs_t[:], 0.0)
        zrhs_t = const_pool.tile([P, 2 * P], bft, tag="zrhs")
        nc.vector.memset(zrhs_t[:], 0.0)

        b_store = persist.tile([P, NT * P], bft, tag="bstore")
        acc_store = persist.tile([P, NT * P], bft, tag="accstore")

        pq_local = dram.tile([NSH, 2 * D_OUT], bft, tag="pq_local")
        d1_local = dram.tile([NSH, D_OUT], bft, tag="d1_local")
        d2_local = dram.tile([NSH, D_OUT], bft, tag="d2_local")
        pq_full = [dram.tile([8 * ROWS_R[r], 2 * D_OUT], bft, tag=f"pq_full{r}",
                             name=f"pq_full{r}", addr_space="Shared")
                   for r in range(4)]
        d1_full = [dram.tile([8 * ROWS_R[r], D_OUT], bft, tag=f"d1_full{r}",
                             name=f"d1_full{r}", addr_space="Shared")
                   for r in range(4)]
        d2_full = [dram.tile([8 * ROWS_R[r], D_OUT], bft, tag=f"d2_full{r}",
                             name=f"d2_full{r}", addr_space="Shared")
                   for r in range(4)]

        def ag_slice(local, full, r):
            nc.gpsimd.collective_compute(
                "AllGather", mybir.AluOpType.bypass, replica_groups=rg,
                ins=[local[OFF_R[r]:OFF_R[r] + ROWS_R[r], :]],
                outs=[full[r][:, :]])

        # ---------- dense phase: pre_sup (feature-major) ----------
        with tc.tile_pool(name="dense", bufs=2) as dense_pool, \
             tc.tile_pool(name="dpsum", bufs=1, space="PSUM") as dpsum_pool:
            for t in range(NT):
                rows = P if t < NT - 1 else LAST_ROWS
                sl = slice(t * P, t * P + rows)
                xt_t = dense_pool.tile([P, KCH * P], fp32, tag="xt")
                for k in range(KCH):
                    nc.sync.dma_start(xt_t[:, k * P:k * P + rows],
                                      xt_d[k * P:(k + 1) * P, sl])
                psa = dpsum_pool.tile([P, P], fp32, tag="pa")
                psb = dpsum_pool.tile([P, P], fp32, tag="pb")
                for k in range(KCH):
                    nc.tensor.matmul(psa[:, :rows],
                                     lhsT=wa_t[:, k * D_OUT:(k + 1) * D_OUT],
                                     rhs=xt_t[:, k * P:k * P + rows],
                                     start=(k == 0), stop=(k == KCH - 1))
                    nc.tensor.matmul(psb[:, :rows],
                                     lhsT=wb_t[:, k * D_OUT:(k + 1) * D_OUT],
                                     rhs=xt_t[:, k * P:k * P + rows],
                                     start=(k == 0), stop=(k == KCH - 1))
                a_sb = dense_pool.tile([P, P], fp32, tag="a_sb")
                nc.vector.tensor_copy(a_sb[:, :rows], psa[:, :rows])
                tmp = dense_pool.tile([P, P], fp32, tag="tmp")
                nc.vector.tensor_tensor(out=tmp[:, :rows], in0=a_sb[:, :rows],
                                        in1=psb[:, :rows], op=AluOp.subtract)
                nc.vector.tensor_tensor(out=tmp[:, :rows], in0=tmp[:, :rows],
                                        in1=a_sb[:, :rows], op=AluOp.mult)
                al_sb = dense_pool.tile([P, P], fp32, tag="al_sb")
                nc.vector.tensor_scalar(out=al_sb[:, :rows], in0=tmp[:, :rows],
                                        scalar1=0.5, scalar2=None,
                                        op0=AluOp.mult)
                nc.vector.tensor_tensor(out=al_sb[:, :rows],
                                        in0=al_sb[:, :rows],
                                        in1=a_sb[:, :rows], op=AluOp.add)
                z = []
                for zi, comp_sb in enumerate((a_sb, al_sb)):
                    psh = dpsum_pool.tile([32, P], fp32, tag="ph")
                    nc.tensor.matmul(psh[:, :rows], lhsT=w1_t[:],
                                     rhs=comp_sb[:, :rows],
                                     start=True, stop=False)
                    nc.tensor.matmul(psh[:, :rows], lhsT=b1_t[:],
                                     rhs=ones_t[:, :rows],
                                     start=False, stop=True)
                    h_sb = dense_pool.tile([32, P], fp32, tag="h_sb")
                    nc.scalar.activation(h_sb[:, :rows], psh[:, :rows],
                                         ActFn.Tanh)
                    psz = dpsum_pool.tile([1, P], fp32, tag="pz")
                    nc.tensor.matmul(psz[:, :rows], lhsT=w2_t[:],
                                     rhs=h_sb[:, :rows], start=True, stop=True)
                    z_sb = dense_pool.tile([1, P], fp32, tag=f"z{zi}")
                    nc.vector.tensor_copy(z_sb[:, :rows], psz[:, :rows])
                    z.append(z_sb)
                dz = dense_pool.tile([1, P], fp32, tag="dz")
                nc.vector.tensor_tensor(out=dz[:, :rows], in0=z[1][:, :rows],
                                        in1=z[0][:, :rows], op=AluOp.subtract)
                ez = dense_pool.tile([1, P], fp32, tag="ez")
                nc.scalar.activation(ez[:, :rows], dz[:, :rows], ActFn.Exp)
                nc.vector.tensor_scalar(out=ez[:, :rows], in0=ez[:, :rows],
                                        scalar1=1.0, scalar2=None,
                                        op0=AluOp.add)
                atta = dense_pool.tile([1, P], fp32, tag="atta")
                nc.vector.reciprocal(atta[:, :rows], ez[:, :rows])
                # broadcast att along partitions via K=1 matmul
                attps = dpsum_pool.tile([P, P], fp32, tag="attps")
                nc.tensor.matmul(attps[:, :rows], lhsT=ones_t[:],
                                 rhs=atta[:, :rows], start=True, stop=True)
                t1 = dense_pool.tile([P, P], fp32, tag="t1")
                nc.vector.tensor_tensor(out=t1[:, :rows], in0=a_sb[:, :rows],
                                        in1=attps[:, :rows], op=AluOp.mult)
                t2 = dense_pool.tile([P, P], fp32, tag="t2")
                nc.vector.tensor_tensor(out=t2[:, :rows], in0=al_sb[:, :rows],
                                        in1=attps[:, :rows], op=AluOp.mult)
                pst = dense_pool.tile([P, P], fp32, tag="pst")
                nc.vector.tensor_tensor(out=pst[:, :rows], in0=al_sb[:, :rows],
                                        in1=t2[:, :rows], op=AluOp.subtract)
                nc.vector.tensor_tensor(out=pst[:, :rows], in0=pst[:, :rows],
                                        in1=t1[:, :rows], op=AluOp.add)
                ptp = dpsum_pool.tile([P, P], fp32, tag="ptp")
                nc.tensor.transpose(out=ptp[:rows, :], in_=pst[:, :rows],
                                    identity=ident[:])
                prow2 = dense_pool.tile([P, 2 * P], bft, tag="prow2")
                nc.vector.tensor_copy(prow2[:rows, 0:P], ptp[:rows, :])
                nc.vector.tensor_tensor(out=prow2[:rows, P:2 * P],
                                        in0=prow2[:rows, 0:P],
                                        in1=prow2[:rows, 0:P], op=AluOp.mult)
                nc.sync.dma_start(pq_local[sl, :], prow2[:rows, :])
                for r in range(4):
                    if t == SLICE_T[r][1] - 1:
                        ag_slice(pq_local, pq_full, r)

        # ---------- SpMM passes ----------
        def run_pass(s, src_full, sink, elem):
            need_q = NEED_Q[s]
            ncol = 2 * P if need_q else P
            with tc.tile_pool(name=f"ps{s}", bufs=2, space="PSUM") as psp:
                for bi, (si_, t0, t1) in enumerate(BLOCKS):
                    pss = [psp.tile([P, ncol], fp32, tag=f"ps{t - t0}",
                                    name=f"ps{t - t0}")
                           for t in range(t0, t1)]
                    for t in range(t0, t1):
                        nc.tensor.matmul(pss[t - t0][:, :], lhsT=zlhs_t[:],
                                         rhs=zrhs_t[:, :ncol],
                                         start=True, stop=False,
                                         skip_group_check=True)
                    # last (region, chunk) per tile for stop flag
                    lastrc = {}
                    for t in range(t0, t1):
                        for r in range(4):
                            if cnts[s, t, r] > 0:
                                lastrc[t] = (r, int(cnts[s, t, r]) - 1)
                    for r in range(4):
                        CNT = cnt_br[(s, bi, r)]
                        if CNT == 0:
                            continue
                        io0 = off_idx[(s, bi, r)]
                        so0 = off_st[(s, bi, r)]
                        SW = stw_br[(s, bi, r)]
                        idxt = meta_pool.tile([P, MAXC * 8], mybir.dt.int16,
                                              tag="idxt")
                        stt = meta_pool.tile([P, MAXW], bft, tag="stt")
                        nc.sync.dma_start(idxt[:, :CNT * 8],
                                          idx_d[:, io0:io0 + CNT * 8])
                        nc.scalar.dma_start(stt[:, :SW],
                                            st_d[:, so0:so0 + SW])
                        gt = g_pool.tile([P, MAXC * 2 * P], bft, tag="G")
                        g3 = gt[:].rearrange("p (c d) -> p c d", d=elem)
                        nc.gpsimd.dma_gather(
                            out_ap=g3[:, 0:CNT, :],
                            in_ap=src_full[r][:, :],
                            idxs_ap=idxt[:, 0:CNT * 8],
                            num_idxs=CNT * P, num_idxs_reg=CNT * P,
                            elem_size=elem, single_packet=False,
                            queue_num=next_q(),
                        )
                        coff = 0
                        soff = 0
                        for t in range(t0, t1):
                            nch = int(cnts[s, t, r])
                            ws = windows[(s, t, r)]
                            for c in range(nch):
                                dlo, w = ws[c]
                                last = lastrc.get(t) == (r, c)
                                nc.tensor.matmul(
                                    pss[t - t0][dlo:dlo + w, :],
                                    lhsT=stt[:, soff:soff + w],
                                    rhs=g3[:, coff + c, 0:ncol],
                                    start=False, stop=last,
                                    skip_group_check=True,
                                    tile_position=(0, dlo))
                                soff += w
                            coff += nch
                    for t in range(t0, t1):
                        sink(t, pss[t - t0])

        def sink_b(t, ps):
            """b_store[:, tile] = s^2 - q (bf16, dest-row-major)."""
            sq = o_pool.tile([P, P], fp32, tag="sq")
            nc.scalar.activation(sq[:], ps[:, 0:P], ActFn.Square)
            nc.vector.tensor_tensor(out=b_store[:, t * P:(t + 1) * P],
                                    in0=sq[:], in1=ps[:, P:2 * P],
                                    op=AluOp.subtract)

        def mk_sink_d(dloc, dfull):
            def sink_d(t, ps):
                rows = P if t < NT - 1 else LAST_ROWS
                sq = o_pool.tile([P, P], fp32, tag="sq")
                nc.scalar.activation(sq[:], ps[:, 0:P], ActFn.Square)
                bb = o_pool.tile([P, P], fp32, tag="bb")
                nc.vector.tensor_tensor(out=bb[:], in0=sq[:],
                                        in1=ps[:, P:2 * P], op=AluOp.subtract)
                drow = o_pool.tile([P, P], bft, tag="drow")
                nc.vector.tensor_tensor(out=drow[:],
                                        in0=b_store[:, t * P:(t + 1) * P],
                                        in1=bb[:], op=AluOp.subtract)
                nc.sync.dma_start(dloc[t * P:t * P + rows, :],
                                  drow[:rows, :])
                for r in range(4):
                    if t == SLICE_T[r][1] - 1:
                        ag_slice(dloc, dfull, r)
            return sink_d

        def sink_acc0(t, ps):
            nc.vector.tensor_copy(acc_store[:, t * P:(t + 1) * P], ps[:, 0:P])

        def sink_acc5(t, ps):
            nc.vector.tensor_tensor(out=acc_store[:, t * P:(t + 1) * P],
                                    in0=acc_store[:, t * P:(t + 1) * P],
                                    in1=ps[:, 0:P], op=AluOp.add)

        def sink_out(t, ps):
            rows = P if t < NT - 1 else LAST_ROWS
            oT = o_pool.tile([P, P], fp32, tag="oT")
            nc.vector.tensor_tensor(out=oT[:], in0=ps[:, 0:P],
                                    in1=acc_store[:, t * P:(t + 1) * P],
                                    op=AluOp.add)
            orow = o_pool.tile([P, P], fp32, tag="orow")
            nc.scalar.activation(orow[:], oT[:], ActFn.Relu)
            nc.sync.dma_start(out_d[t * P:t * P + rows, :], orow[:rows, :])

        run_pass(1, pq_full, sink_b, 2 * P)
        run_pass(3, pq_full, mk_sink_d(d1_local, d1_full), 2 * P)
        run_pass(2, pq_full, sink_b, 2 * P)
        run_pass(4, pq_full, mk_sink_d(d2_local, d2_full), 2 * P)
        run_pass(0, pq_full, sink_acc0, 2 * P)
        run_pass(5, d1_full, sink_acc5, P)
        run_pass(6, d2_full, sink_out, P)

    nc.compile()
    return nc


def kernel(x, Wa, Wb, Wc, attn_w1, attn_b1, attn_w2, rows, cols, vals):
    from concourse.bass_utils import run_bass_kernel_spmd

    x = np.asarray(x, np.float32)
    Wa = np.asarray(Wa, np.float32)
    Wb = np.asarray(Wb, np.float32)
    attn_w1 = np.asarray(attn_w1, np.float32)
    attn_b1 = np.asarray(attn_b1, np.float32)
    attn_w2 = np.asarray(attn_w2, np.float32)
    rows = np.asarray(rows)
    cols = np.asarray(cols)
    vals = np.asarray(vals, np.float32)

    cnts, windows, off_idx, off_st, idx_w, st_w, in_meta = _build_meta(
        rows, cols, vals)

    in_maps = []
    for m in range(NCORE):
        idx_all, st_all = in_meta[m]
        xt = np.ascontiguousarray(x[m * NSH:(m + 1) * NSH, :].T)
        in_maps.append({
            "xt": xt, "wa": Wa, "wb": Wb, "w1": attn_w1,
            "b1": attn_b1.reshape(1, 32), "w2": attn_w2,
            "idxm": idx_all, "stm": st_all,
        })

    nc = _build_program(cnts, windows, off_idx, off_st, idx_w, st_w)
    res = run_bass_kernel_spmd(nc, in_maps, core_ids=list(range(NCORE)))
    out = np.concatenate([res.results[m]["out"] for m in range(NCORE)], axis=0)
    return np.ascontiguousarray(out.astype(np.float32))


# revision 7
# speedup vs baseline: 1.9193x; 1.5764x over previous
"""Trainium2 Bass kernel for nn_BGraphConvolution (BGCN message passing).

v3 design (from v2 trace analysis: Pool engine was the critical path at 92%
coverage; dma_gather descriptor-gen is ~2.5ns/idx intrinsic and per-index
bound; DVE per-chunk one-hot builds and their port contention with GpSimd
made everything 3x worse):
- scatter matrices (val-scaled one-hots) precomputed on HOST and DMA'd in;
  no DVE work per chunk.
- flipped matmul orientation: lhsT = scatter chunk [128 slots, w dests]
  (stationary, cheap LDWEIGHTS), rhs = gathered chunk (streaming). Output
  psum is dest-row-major -> no output transposes.
- 256B gathers of p rows; for the q-passes the gathered block is squared
  on ACT into the second half of the same SBUF tile and ONE fused N=256
  matmul per chunk (2-dim moving AP spanning both halves) produces both
  the s-SpMM and q-SpMM columns of psum.
- dense chunk packing (chunks of 128 edge slots, dest-sorted, rows may
  split across chunks), PSUM windows aligned per PE tile_position rules.
- AllGathers sliced 4x, overlapped with the dense phase / d-pass sinks.
"""
import os
import numpy as np
import ml_dtypes

N = 100000
D_IN, D_OUT = 256, 128
NCORE = 8
NSH = N // NCORE          # 12500 rows per core
P = 128
NT = (NSH + P - 1) // P   # 98 dest tiles (last has 84 rows)
LAST_ROWS = NSH - (NT - 1) * P
NQ = 4

ROWS_R = [3200, 3200, 3200, 2900]   # owner-local rows per AG slice/region
OFF_R = [0, 3200, 6400, 9600]
SLICE_T = [(0, 25), (25, 50), (50, 75), (75, 98)]  # dest-tile slices
BMAX = int(os.environ.get("BGCN_BMAX", "4"))
BLOCKS = []          # (slice_idx, t0, t1)
for _si, (_a, _b) in enumerate(SLICE_T):
    _t = _a
    while _t < _b:
        _e = min(_t + BMAX, _b)
        BLOCKS.append((_si, _t, _e))
        _t = _e
PASS_ORDER = [1, 3, 2, 4, 0, 5, 6]
NEED_Q = {1: True, 3: True, 2: True, 4: True, 0: False, 5: False, 6: False}
VSCALE = [0.5, 1.0, 1.0, 1.0, 1.0, 0.125, 0.125]

bf16 = ml_dtypes.bfloat16


def _aligned_window(dmin, dmax):
    """Smallest PE-legal psum window [dlo, dlo+w) covering [dmin, dmax).

    Legal: w<=32 at dlo in {0,32,64,96}; w<=64 at dlo in {0,64}; else dlo=0.
    """
    b32 = dmin // 32
    if dmax <= (b32 + 1) * 32:
        return b32 * 32, dmax - b32 * 32
    b64 = dmin // 64
    if dmax <= (b64 + 1) * 64:
        return b64 * 64, dmax - b64 * 64
    return 0, dmax


def _build_meta(rows, cols, vals):
    """Host preprocessing.

    Returns (cnts, windows, off_idx, off_st, idx_w, st_w, in_meta) where
    cnts[s,t,r] = shared chunk count, windows[(s,t,r)] = [(dlo,w), ...],
    and in_meta[m] = (idx_all [128, idx_w] int16, st_all [128, st_w] bf16).
    """
    rows_r = np.array(ROWS_R, np.int64)
    off_r = np.array(OFF_R, np.int64)

    # per (m, s): sorted edge arrays + per-(t,r) boundaries
    per = {}
    for m in range(NCORE):
        lo, hi = m * NSH, (m + 1) * NSH
        for s in range(7):
            mask = (rows[s] >= lo) & (rows[s] < hi)
            rl = (rows[s][mask] - lo).astype(np.int64)
            cl = cols[s][mask].astype(np.int64)
            vl = (vals[s][mask] * VSCALE[s]).astype(np.float32)
            t_arr = rl // P
            d_arr = rl % P
            own = cl // NSH
            loc = cl % NSH
            r_arr = np.minimum(loc // 3200, 3)
            idx_arr = own * rows_r[r_arr] + (loc - off_r[r_arr])
            order = np.lexsort((idx_arr, d_arr, r_arr, t_arr))
            key = t_arr[order] * 4 + r_arr[order]
            bounds = np.searchsorted(key, np.arange(NT * 4 + 1))
            per[(m, s)] = (d_arr[order], idx_arr[order], vl[order], bounds)

    # shared chunk counts
    cnts = np.zeros((7, NT, 4), np.int32)
    for (m, s), (_, _, _, bounds) in per.items():
        ne = bounds[1:] - bounds[:-1]
        nch = -(-ne // P)
        cnts[s] = np.maximum(cnts[s], nch.reshape(NT, 4))
    cnts = np.maximum(cnts, 1)  # keep zero-init/window structure simple

    # shared windows: union over cores of each chunk's dest span
    windows = {}
    for s in range(7):
        for t in range(NT):
            for r in range(4):
                nc_ = int(cnts[s, t, r])
                lo_ = np.full(nc_, P, np.int64)
                hi_ = np.zeros(nc_, np.int64)
                for m in range(NCORE):
                    d_s, _, _, bounds = per[(m, s)]
                    b0, b1 = bounds[t * 4 + r], bounds[t * 4 + r + 1]
                    if b0 == b1:
                        continue
                    dd = d_s[b0:b1]
                    for c in range(-(-(b1 - b0) // P)):
                        seg = dd[c * P:(c + 1) * P]
                        lo_[c] = min(lo_[c], int(seg[0]))
                        hi_[c] = max(hi_[c], int(seg[-1]) + 1)
                ws = []
                for c in range(nc_):
                    if hi_[c] <= lo_[c]:
                        ws.append((0, 4))
                    else:
                        ws.append(_aligned_window(int(lo_[c]), int(hi_[c])))
                windows[(s, t, r)] = ws

    # shared segment offsets: order = PASS_ORDER x BLOCKS x regions
    off_idx, off_st = {}, {}
    io = so = 0
    for s in PASS_ORDER:
        for bi, (_si, t0, t1) in enumerate(BLOCKS):
            for r in range(4):
                off_idx[(s, bi, r)] = io
                off_st[(s, bi, r)] = so
                io += int(cnts[s, t0:t1, r].sum()) * 8
                for t in range(t0, t1):
                    for (_dlo, w) in windows[(s, t, r)]:
                        so += w
    idx_w, st_w = io, so

    # per-core packed arrays
    in_meta = []
    for m in range(NCORE):
        idx_all = np.zeros((P, idx_w), np.int16)
        st_all = np.zeros((P, st_w), bf16)
        st_f32 = np.zeros((P, st_w), np.float32)
        for s in PASS_ORDER:
            d_s, i_s, v_s, bounds = per[(m, s)]
            for bi, (_si, t0, t1) in enumerate(BLOCKS):
                for r in range(4):
                    CNT = int(cnts[s, t0:t1, r].sum())
                    iseg = np.zeros(CNT * P, np.int16)
                    coff = 0
                    soff = off_st[(s, bi, r)]
                    for t in range(t0, t1):
                        b0, b1 = bounds[t * 4 + r], bounds[t * 4 + r + 1]
                        ne = b1 - b0
                        nch = int(cnts[s, t, r])
                        if ne > 0:
                            iseg[coff * P:coff * P + ne] = i_s[b0:b1]
                            ws = windows[(s, t, r)]
                            # scatter vals into st columns
                            e = np.arange(ne)
                            ch = e // P
                            slot = e % P
                            dlo_arr = np.array([ws[c][0] for c in ch])
                            w_cum = np.concatenate(
                                ([0], np.cumsum([w for (_dl, w) in ws])))
                            col = soff + w_cum[ch] + (d_s[b0:b1] - dlo_arr)
                            st_f32[slot, col] = v_s[b0:b1]
                        coff += nch
                        soff += sum(w for (_dl, w) in windows[(s, t, r)])
                    io0 = off_idx[(s, bi, r)]
                    idx_all[:, io0:io0 + CNT * 8] = np.tile(
                        iseg.reshape(CNT * 8, 16).T, (8, 1))
        st_all[:] = st_f32.astype(bf16)
        in_meta.append((idx_all, st_all))
    return cnts, windows, off_idx, off_st, idx_w, st_w, in_meta


def _build_program(cnts, windows, off_idx, off_st, idx_w, st_w):
    import concourse.bass as bass
    import concourse.tile as tile
    from concourse import bacc, mybir, library_config
    from concourse.masks import make_identity
    from contextlib import ExitStack

    fp32 = mybir.dt.float32
    bft = mybir.dt.bfloat16
    KCH = D_IN // P  # 2
    AluOp = mybir.AluOpType
    ActFn = mybir.ActivationFunctionType

    cnt_br = {}
    for s in range(7):
        for bi, (_si, t0, t1) in enumerate(BLOCKS):
            for r in range(4):
                cnt_br[(s, bi, r)] = int(cnts[s, t0:t1, r].sum())
    MAXC = max(cnt_br.values())
    stw_br = {}
    for s in range(7):
        for bi, (_si, t0, t1) in enumerate(BLOCKS):
            for r in range(4):
                stw_br[(s, bi, r)] = sum(
                    w for t in range(t0, t1) for (_dl, w) in windows[(s, t, r)])
    MAXW = max(stw_br.values())

    nc = bacc.Bacc("TRN2", target_bir_lowering=False, debug=False,
                   num_devices=NCORE, num_swdge_queues=NQ,
                   dynamic_dma_scratch_size=49152)
    xt_d = nc.dram_tensor("xt", [D_IN, NSH], fp32, kind="ExternalInput").ap()
    wa_d = nc.dram_tensor("wa", [D_IN, D_OUT], fp32, kind="ExternalInput").ap()
    wb_d = nc.dram_tensor("wb", [D_IN, D_OUT], fp32, kind="ExternalInput").ap()
    w1_d = nc.dram_tensor("w1", [D_OUT, 32], fp32, kind="ExternalInput").ap()
    b1_d = nc.dram_tensor("b1", [1, 32], fp32, kind="ExternalInput").ap()
    w2_d = nc.dram_tensor("w2", [32, 1], fp32, kind="ExternalInput").ap()
    idx_d = nc.dram_tensor("idxm", [P, idx_w], mybir.dt.int16,
                           kind="ExternalInput").ap()
    st_d = nc.dram_tensor("stm", [P, st_w], bft, kind="ExternalInput").ap()
    out_d = nc.dram_tensor("out", [NSH, D_OUT], fp32, kind="ExternalOutput").ap()

    qctr = [0]

    def next_q():
        q = qctr[0] % NQ
        qctr[0] += 1
        return q

    rg = [list(range(NCORE))]

    with tile.TileContext(nc) as tc, ExitStack() as ctx:
        const_pool = ctx.enter_context(tc.tile_pool(name="const", bufs=1))
        persist = ctx.enter_context(tc.tile_pool(name="persist", bufs=1))
        meta_pool = ctx.enter_context(tc.tile_pool(name="meta", bufs=4))
        g_pool = ctx.enter_context(tc.tile_pool(name="g", bufs=4))
        o_pool = ctx.enter_context(tc.tile_pool(name="o", bufs=6))
        dram = ctx.enter_context(tc.tile_pool(name="dram", bufs=1, space="DRAM"))

        nc.gpsimd.load_library(library_config.mlp)

        ident = const_pool.tile([P, P], fp32)
        make_identity(nc, ident[:])
        wa_t = const_pool.tile([P, KCH * D_OUT], fp32, tag="wa")
        wb_t = const_pool.tile([P, KCH * D_OUT], fp32, tag="wb")
        for k in range(KCH):
            nc.sync.dma_start(wa_t[:, k * D_OUT:(k + 1) * D_OUT],
                              wa_d[k * P:(k + 1) * P, :])
            nc.sync.dma_start(wb_t[:, k * D_OUT:(k + 1) * D_OUT],
                              wb_d[k * P:(k + 1) * P, :])
        w1_t = const_pool.tile([P, 32], fp32)
        nc.sync.dma_start(w1_t[:], w1_d[:])
        b1_t = const_pool.tile([1, 32], fp32)
        nc.sync.dma_start(b1_t[:], b1_d[:])
        w2_t = const_pool.tile([32, 1], fp32)
        nc.sync.dma_start(w2_t[:], w2_d[:])
        ones_t = const_pool.tile([1, P], fp32)
        nc.vector.memset(ones_t[:], 1.0)
        zlhs_t = const_pool.tile([P, P], bft, tag="zlhs")
        nc.vector.memset(zlhs_t[:], 0.0)
        zrhs_t = const_pool.tile([P, 2 * P], bft, tag="zrhs")
        nc.vector.memset(zrhs_t[:], 0.0)

        b_store = persist.tile([P, NT * P], bft, tag="bstore")
        acc_store = persist.tile([P, NT * P], bft, tag="accstore")

        p_local = dram.tile([NSH, D_OUT], bft, tag="p_local")
        d1_local = dram.tile([NSH, D_OUT], bft, tag="d1_local")
        d2_local = dram.tile([NSH, D_OUT], bft, tag="d2_local")
        p_full = [dram.tile([8 * ROWS_R[r], D_OUT], bft, tag=f"p_full{r}",
                            name=f"p_full{r}", addr_space="Shared")
                  for r in range(4)]
        d1_full = [dram.tile([8 * ROWS_R[r], D_OUT], bft, tag=f"d1_full{r}",
                             name=f"d1_full{r}", addr_space="Shared")
                   for r in range(4)]
        d2_full = [dram.tile([8 * ROWS_R[r], D_OUT], bft, tag=f"d2_full{r}",
                             name=f"d2_full{r}", addr_space="Shared")
                   for r in range(4)]

        def ag_slice(local, full, r):
            nc.gpsimd.collective_compute(
                "AllGather", mybir.AluOpType.bypass, replica_groups=rg,
                ins=[local[OFF_R[r]:OFF_R[r] + ROWS_R[r], :]],
                outs=[full[r][:, :]])

        # ---------- dense phase: pre_sup (feature-major) ----------
        with tc.tile_pool(name="dense", bufs=2) as dense_pool, \
             tc.tile_pool(name="dpsum", bufs=1, space="PSUM") as dpsum_pool:
            for t in range(NT):
                rows = P if t < NT - 1 else LAST_ROWS
                sl = slice(t * P, t * P + rows)
                xt_t = dense_pool.tile([P, KCH * P], fp32, tag="xt")
                for k in range(KCH):
                    nc.sync.dma_start(xt_t[:, k * P:k * P + rows],
                                      xt_d[k * P:(k + 1) * P, sl])
                psa = dpsum_pool.tile([P, P], fp32, tag="pa")
                psb = dpsum_pool.tile([P, P], fp32, tag="pb")
                for k in range(KCH):
                    nc.tensor.matmul(psa[:, :rows],
                                     lhsT=wa_t[:, k * D_OUT:(k + 1) * D_OUT],
                                     rhs=xt_t[:, k * P:k * P + rows],
                                     start=(k == 0), stop=(k == KCH - 1))
                    nc.tensor.matmul(psb[:, :rows],
                                     lhsT=wb_t[:, k * D_OUT:(k + 1) * D_OUT],
                                     rhs=xt_t[:, k * P:k * P + rows],
                                     start=(k == 0), stop=(k == KCH - 1))
                a_sb = dense_pool.tile([P, P], fp32, tag="a_sb")
                nc.vector.tensor_copy(a_sb[:, :rows], psa[:, :rows])
                tmp = dense_pool.tile([P, P], fp32, tag="tmp")
                nc.vector.tensor_tensor(out=tmp[:, :rows], in0=a_sb[:, :rows],
                                        in1=psb[:, :rows], op=AluOp.subtract)
                nc.vector.tensor_tensor(out=tmp[:, :rows], in0=tmp[:, :rows],
                                        in1=a_sb[:, :rows], op=AluOp.mult)
                al_sb = dense_pool.tile([P, P], fp32, tag="al_sb")
                nc.vector.tensor_scalar(out=al_sb[:, :rows], in0=tmp[:, :rows],
                                        scalar1=0.5, scalar2=None,
                                        op0=AluOp.mult)
                nc.vector.tensor_tensor(out=al_sb[:, :rows],
                                        in0=al_sb[:, :rows],
                                        in1=a_sb[:, :rows], op=AluOp.add)
                z = []
                for zi, comp_sb in enumerate((a_sb, al_sb)):
                    psh = dpsum_pool.tile([32, P], fp32, tag="ph")
                    nc.tensor.matmul(psh[:, :rows], lhsT=w1_t[:],
                                     rhs=comp_sb[:, :rows],
                                     start=True, stop=False)
                    nc.tensor.matmul(psh[:, :rows], lhsT=b1_t[:],
                                     rhs=ones_t[:, :rows],
                                     start=False, stop=True)
                    h_sb = dense_pool.tile([32, P], fp32, tag="h_sb")
                    nc.scalar.activation(h_sb[:, :rows], psh[:, :rows],
                                         ActFn.Tanh)
                    psz = dpsum_pool.tile([1, P], fp32, tag="pz")
                    nc.tensor.matmul(psz[:, :rows], lhsT=w2_t[:],
                                     rhs=h_sb[:, :rows], start=True, stop=True)
                    z_sb = dense_pool.tile([1, P], fp32, tag=f"z{zi}")
                    nc.vector.tensor_copy(z_sb[:, :rows], psz[:, :rows])
                    z.append(z_sb)
                dz = dense_pool.tile([1, P], fp32, tag="dz")
                nc.vector.tensor_tensor(out=dz[:, :rows], in0=z[1][:, :rows],
                                        in1=z[0][:, :rows], op=AluOp.subtract)
                ez = dense_pool.tile([1, P], fp32, tag="ez")
                nc.scalar.activation(ez[:, :rows], dz[:, :rows], ActFn.Exp)
                nc.vector.tensor_scalar(out=ez[:, :rows], in0=ez[:, :rows],
                                        scalar1=1.0, scalar2=None,
                                        op0=AluOp.add)
                atta = dense_pool.tile([1, P], fp32, tag="atta")
                nc.vector.reciprocal(atta[:, :rows], ez[:, :rows])
                # broadcast att along partitions via K=1 matmul
                attps = dpsum_pool.tile([P, P], fp32, tag="attps")
                nc.tensor.matmul(attps[:, :rows], lhsT=ones_t[:],
                                 rhs=atta[:, :rows], start=True, stop=True)
                t1 = dense_pool.tile([P, P], fp32, tag="t1")
                nc.vector.tensor_tensor(out=t1[:, :rows], in0=a_sb[:, :rows],
                                        in1=attps[:, :rows], op=AluOp.mult)
                t2 = dense_pool.tile([P, P], fp32, tag="t2")
                nc.vector.tensor_tensor(out=t2[:, :rows], in0=al_sb[:, :rows],
                                        in1=attps[:, :rows], op=AluOp.mult)
                pst = dense_pool.tile([P, P], fp32, tag="pst")
                nc.vector.tensor_tensor(out=pst[:, :rows], in0=al_sb[:, :rows],
                                        in1=t2[:, :rows], op=AluOp.subtract)
                nc.vector.tensor_tensor(out=pst[:, :rows], in0=pst[:, :rows],
                                        in1=t1[:, :rows], op=AluOp.add)
                ptp = dpsum_pool.tile([P, P], fp32, tag="ptp")
                nc.tensor.transpose(out=ptp[:rows, :], in_=pst[:, :rows],
                                    identity=ident[:])
                prow = dense_pool.tile([P, P], bft, tag="prow")
                nc.vector.tensor_copy(prow[:rows, :], ptp[:rows, :])
                nc.sync.dma_start(p_local[sl, :], prow[:rows, :])
                for r in range(4):
                    if t == SLICE_T[r][1] - 1:
                        ag_slice(p_local, p_full, r)

        # ---------- SpMM passes ----------
        def run_pass(s, src_full, sink):
            need_q = NEED_Q[s]
            ncol = 2 * P if need_q else P
            with tc.tile_pool(name=f"ps{s}", bufs=2, space="PSUM") as psp:
                for bi, (si_, t0, t1) in enumerate(BLOCKS):
                    pss = [psp.tile([P, ncol], fp32, tag=f"ps{t - t0}",
                                    name=f"ps{t - t0}")
                           for t in range(t0, t1)]
                    for t in range(t0, t1):
                        nc.tensor.matmul(pss[t - t0][:, :], lhsT=zlhs_t[:],
                                         rhs=zrhs_t[:, :ncol],
                                         start=True, stop=False,
                                         skip_group_check=True)
                    # last (region, chunk) per tile for stop flag
                    lastrc = {}
                    for t in range(t0, t1):
                        for r in range(4):
                            if cnts[s, t, r] > 0:
                                lastrc[t] = (r, int(cnts[s, t, r]) - 1)
                    for r in range(4):
                        CNT = cnt_br[(s, bi, r)]
                        if CNT == 0:
                            continue
                        io0 = off_idx[(s, bi, r)]
                        so0 = off_st[(s, bi, r)]
                        SW = stw_br[(s, bi, r)]
                        idxt = meta_pool.tile([P, MAXC * 8], mybir.dt.int16,
                                              tag="idxt")
                        stt = meta_pool.tile([P, MAXW], bft, tag="stt")
                        nc.sync.dma_start(idxt[:, :CNT * 8],
                                          idx_d[:, io0:io0 + CNT * 8])
                        nc.scalar.dma_start(stt[:, :SW],
                                            st_d[:, so0:so0 + SW])
                        gt = g_pool.tile([P, MAXC * 2 * P], bft, tag="G")
                        g3 = gt[:].rearrange("p (c d) -> p c d", d=P)
                        nc.gpsimd.dma_gather(
                            out_ap=g3[:, 0:CNT, :],
                            in_ap=src_full[r][:, :],
                            idxs_ap=idxt[:, 0:CNT * 8],
                            num_idxs=CNT * P, num_idxs_reg=CNT * P,
                            elem_size=P, single_packet=False,
                            queue_num=next_q(),
                        )
                        if need_q:
                            # square gathered rows into the second half of gt
                            nc.scalar.activation(
                                gt[:, CNT * P:2 * CNT * P],
                                gt[:, 0:CNT * P], ActFn.Square)
                            gq4 = gt[:, 0:2 * CNT * P].rearrange(
                                "p (h c d) -> p h c d", h=2, d=P)
                        coff = 0
                        soff = 0
                        for t in range(t0, t1):
                            nch = int(cnts[s, t, r])
                            ws = windows[(s, t, r)]
                            for c in range(nch):
                                dlo, w = ws[c]
                                last = lastrc.get(t) == (r, c)
                                if need_q:
                                    rhs = gq4[:, :, coff + c, :]
                                else:
                                    rhs = g3[:, coff + c, :]
                                nc.tensor.matmul(
                                    pss[t - t0][dlo:dlo + w, :],
                                    lhsT=stt[:, soff:soff + w],
                                    rhs=rhs,
                                    start=False, stop=last,
                                    skip_group_check=True,
                                    tile_position=(0, dlo))
                                soff += w
                            coff += nch
                    for t in range(t0, t1):
                        sink(t, pss[t - t0])

        def sink_b(t, ps):
            """b_store[:, tile] = s^2 - q (bf16, dest-row-major)."""
            sq = o_pool.tile([P, P], fp32, tag="sq")
            nc.scalar.activation(sq[:], ps[:, 0:P], ActFn.Square)
            nc.vector.tensor_tensor(out=b_store[:, t * P:(t + 1) * P],
                                    in0=sq[:], in1=ps[:, P:2 * P],
                                    op=AluOp.subtract)

        def mk_sink_d(dloc, dfull):
            def sink_d(t, ps):
                rows = P if t < NT - 1 else LAST_ROWS
                sq = o_pool.tile([P, P], fp32, tag="sq")
                nc.scalar.activation(sq[:], ps[:, 0:P], ActFn.Square)
                bb = o_pool.tile([P, P], fp32, tag="bb")
                nc.vector.tensor_tensor(out=bb[:], in0=sq[:],
                                        in1=ps[:, P:2 * P], op=AluOp.subtract)
                drow = o_pool.tile([P, P], bft, tag="drow")
                nc.vector.tensor_tensor(out=drow[:],
                                        in0=b_store[:, t * P:(t + 1) * P],
                                        in1=bb[:], op=AluOp.subtract)
                nc.sync.dma_start(dloc[t * P:t * P + rows, :],
                                  drow[:rows, :])
                for r in range(4):
                    if t == SLICE_T[r][1] - 1:
                        ag_slice(dloc, dfull, r)
            return sink_d

        def sink_acc0(t, ps):
            nc.vector.tensor_copy(acc_store[:, t * P:(t + 1) * P], ps[:, 0:P])

        def sink_acc5(t, ps):
            nc.vector.tensor_tensor(out=acc_store[:, t * P:(t + 1) * P],
                                    in0=acc_store[:, t * P:(t + 1) * P],
                                    in1=ps[:, 0:P], op=AluOp.add)

        def sink_out(t, ps):
            rows = P if t < NT - 1 else LAST_ROWS
            oT = o_pool.tile([P, P], fp32, tag="oT")
            nc.vector.tensor_tensor(out=oT[:], in0=ps[:, 0:P],
                                    in1=acc_store[:, t * P:(t + 1) * P],
                                    op=AluOp.add)
            orow = o_pool.tile([P, P], fp32, tag="orow")
            nc.scalar.activation(orow[:], oT[:], ActFn.Relu)
            nc.sync.dma_start(out_d[t * P:t * P + rows, :], orow[:rows, :])

        run_pass(1, p_full, sink_b)
        run_pass(3, p_full, mk_sink_d(d1_local, d1_full))
        run_pass(2, p_full, sink_b)
        run_pass(4, p_full, mk_sink_d(d2_local, d2_full))
        run_pass(0, p_full, sink_acc0)
        run_pass(5, d1_full, sink_acc5)
        run_pass(6, d2_full, sink_out)

    nc.compile()
    return nc


def kernel(x, Wa, Wb, Wc, attn_w1, attn_b1, attn_w2, rows, cols, vals):
    from concourse.bass_utils import run_bass_kernel_spmd

    x = np.asarray(x, np.float32)
    Wa = np.asarray(Wa, np.float32)
    Wb = np.asarray(Wb, np.float32)
    attn_w1 = np.asarray(attn_w1, np.float32)
    attn_b1 = np.asarray(attn_b1, np.float32)
    attn_w2 = np.asarray(attn_w2, np.float32)
    rows = np.asarray(rows)
    cols = np.asarray(cols)
    vals = np.asarray(vals, np.float32)

    cnts, windows, off_idx, off_st, idx_w, st_w, in_meta = _build_meta(
        rows, cols, vals)

    in_maps = []
    for m in range(NCORE):
        idx_all, st_all = in_meta[m]
        xt = np.ascontiguousarray(x[m * NSH:(m + 1) * NSH, :].T)
        in_maps.append({
            "xt": xt, "wa": Wa, "wb": Wb, "w1": attn_w1,
            "b1": attn_b1.reshape(1, 32), "w2": attn_w2,
            "idxm": idx_all, "stm": st_all,
        })

    nc = _build_program(cnts, windows, off_idx, off_st, idx_w, st_w)
    res = run_bass_kernel_spmd(nc, in_maps, core_ids=list(range(NCORE)))
    out = np.concatenate([res.results[m]["out"] for m in range(NCORE)], axis=0)
    return np.ascontiguousarray(out.astype(np.float32))
